# revision 29
# baseline (speedup 1.0000x reference)
"""Trainium2 Bass kernel for nn_DisentangledVAE (9-layer coupling flow + 2-layer LSTM prior).

Self-contained: builds per-core feature-major layout on host, runs SPMD on 8 NeuronCores.
Flow matmuls in float32r (full PE rate at N=512, ~1.4e-4 err), LSTM in bf16 (loss-weight 0.0025).
"""
import os
import sys

sys.path.insert(0, '/opt/trn_rl_repo')
import numpy as np
import concourse.bass as bass  # noqa: F401
from concourse.bacc import Bacc
import concourse.tile as tile
from concourse import mybir
from concourse.bass_utils import run_bass_kernel_spmd

f32 = mybir.dt.float32
f32r = mybir.dt.float32r
bf16 = mybir.dt.bfloat16
AF = mybir.ActivationFunctionType
ALU = mybir.AluOpType
AX = mybir.AxisListType

Z = 256
T = 16
NF = 512
H = 512
NL = 9
B = 1024
NCORES = 8
BL = B // NCORES          # 128 sequences per core
C = BL * T                # 2048 columns per core (col = s*16 + t)
CH = 512                  # N-chunk
NCH = C // CH
LOG_SQRT_2PI = 0.9189385332046727

EV = np.arange(0, Z, 2)
OD = np.arange(1, Z, 2)
PERM_EO = np.concatenate([EV, OD])

# flow weight blob (f32r): 8 a/b layers (8192 cols each) + 'c' piece1 (8192) + 'c' piece2 (2048)
WAB_COLS = 8192
# LSTM blob (bf16) col offsets
OW_IH1, OW_HH1, OW_IH2, OW_HH2, OW_ML, OB1, OB2, OBML = (
    0, 4096, 12288, 20480, 28672, 30720, 32768, 34816)
WL_COLS = 35328

_CACHE = {}


def _layer_masks(i):
    # i odd -> mask 'a' (odd channels kept); i even>=2 -> 'b' (even kept); i==0 -> 'c'
    if i == 0:
        return None, None
    if i % 2 == 1:
        return OD, EV   # kept, updated
    return EV, OD


def _build_nc():
    nc = Bacc()
    d = {}
    for nm in ("xe", "xo", "pe", "po"):
        d[nm] = nc.declare_dram_parameter(nm, [128, C], f32r, isOutput=False)
    d["wf"] = nc.declare_dram_parameter("wf", [128, 10 * WAB_COLS], f32r, isOutput=False)
    d["bf"] = nc.declare_dram_parameter("bf", [128, NL * 20], f32, isOutput=False)
    d["wl"] = nc.declare_dram_parameter("wl", [128, WL_COLS], bf16, isOutput=False)
    d["z3e"] = nc.declare_dram_parameter("z3e", [128, C], f32r, isOutput=True)
    d["z3o"] = nc.declare_dram_parameter("z3o", [128, C], f32r, isOutput=True)
    d["accs"] = nc.declare_dram_parameter("accs", [128, 4], f32, isOutput=True)

    from contextlib import ExitStack
    with tile.TileContext(nc) as tc, ExitStack() as top:
        pers = top.enter_context(tc.tile_pool(name="pers", bufs=1))
        zpool = top.enter_context(tc.tile_pool(name="zp", bufs=2))

        pe_t = pers.tile([128, C], f32r, tag="pe")
        po_t = pers.tile([128, C], f32r, tag="po")
        for lo, hi in ((0, C // 2), (C // 2, C)):
            nc.sync.dma_start(out=pe_t[:, lo:hi], in_=d["pe"][:, lo:hi])
            nc.sync.dma_start(out=po_t[:, lo:hi], in_=d["po"][:, lo:hi])
        acc = pers.tile([128, 4], f32, tag="acc")
        nc.vector.memset(acc, 0.0)
        rtmp = pers.tile([128, 1], f32, tag="rtmp")
        zfin = [None, None]

        ze = zpool.tile([128, C], f32r, tag="ze")
        zo = zpool.tile([128, C], f32r, tag="zo")
        for lo, hi in ((0, C // 2), (C // 2, C)):
            nc.sync.dma_start(out=ze[:, lo:hi], in_=d["xe"][:, lo:hi])

        # cell1 x-weights+bias: no-dependency DMA at kernel start, hidden under flow
        wlp = top.enter_context(tc.tile_pool(name="wlp", bufs=1))
        wla = wlp.tile([128, 6144], bf16, tag="wla")
        wlbx = wlp.tile([128, 10240], bf16, tag="wlbx")
        ones = wlp.tile([128, CH], bf16, tag="ones")
        nc.vector.memset(ones, 1.0)

        # ---------------- flow ----------------
        with ExitStack() as fs:
            wpool = fs.enter_context(tc.tile_pool(name="wp", bufs=2))
            bpool = fs.enter_context(tc.tile_pool(name="bp", bufs=2))
            h1pool = fs.enter_context(tc.tile_pool(name="h1p", bufs=1))
            h2pool = fs.enter_context(tc.tile_pool(name="h2p", bufs=1))
            stpool = fs.enter_context(tc.tile_pool(name="stp", bufs=2))
            mmp = fs.enter_context(tc.tile_pool(name="mmp", bufs=4, space="PSUM"))
            zshp = fs.enter_context(tc.tile_pool(name="zshp", bufs=1))

            def _mk_zsh(srct, name):
                dst = zshp.tile([128, C], f32r, tag=name, name=name)
                s3 = srct.rearrange("p (s t) -> p s t", t=T)
                d3 = dst.rearrange("p (s t) -> p s t", t=T)
                nc.vector.tensor_copy(out=d3[:, :, 1:T], in_=s3[:, :, 0:T - 1])
                nc.vector.memset(d3[:, :, 0:1].bitcast(f32), 0.0)
                return dst

            for j, li in enumerate([8, 7, 6, 5, 4, 3, 2, 1, 0]):
                isc = (li == 0)
                base = j * WAB_COLS
                w1t = wpool.tile([128, 4096], f32r, tag="w1", bufs=1)
                w23 = wpool.tile([128, 6144], f32r, tag="w23")
                if isc:
                    nc.sync.dma_start(out=w1t[:, 0:4096], in_=d["wf"][:, base:base + 4096])
                    nc.sync.dma_start(out=w23[:, 0:4096], in_=d["wf"][:, base + 4096:base + 8192])
                    nc.sync.dma_start(out=w23[:, 4096:6144],
                                      in_=d["wf"][:, 9 * WAB_COLS:9 * WAB_COLS + 2048])
                elif j == 0:
                    for lo, hi in ((0, 1536), (1536, 3072)):
                        nc.sync.dma_start(out=w1t[:, lo:hi], in_=d["wf"][:, base + lo:base + hi])
                    nc.sync.dma_start(out=w23[:, 0:5120], in_=d["wf"][:, base + 3072:base + 8192])
                    for lo, hi in ((0, C // 2), (C // 2, C)):
                        nc.sync.dma_start(out=zo[:, lo:hi], in_=d["xo"][:, lo:hi])
                else:
                    nc.sync.dma_start(out=w1t[:, 0:3072], in_=d["wf"][:, base:base + 3072])
                    nc.sync.dma_start(out=w23[:, 0:5120], in_=d["wf"][:, base + 3072:base + 8192])
                bt = bpool.tile([128, 20], f32, tag="b")
                nc.sync.dma_start(out=bt, in_=d["bf"][:, j * 20:(j + 1) * 20])
                if j == 0:
                    nc.sync.dma_start(out=wla[:, 0:4096], in_=d["wl"][:, 0:4096])
                    nc.sync.dma_start(out=wla[:, 4096:6144], in_=d["wl"][:, OB1:OB1 + 2048])
                    nc.sync.dma_start(out=wlbx[:, 0:8192], in_=d["wl"][:, OW_IH2:OW_IH2 + 8192])
                    nc.sync.dma_start(out=wlbx[:, 8192:10240], in_=d["wl"][:, OB2:OB2 + 2048])

                if isc:
                    inkts = [zsh_e, zsh_o, pe_t, po_t]
                    w1off, w2soff, w2toff = 0, 0, 2048
                    w3soff, w3toff = 4096, 5120
                    n3 = 2  # L3 mtiles per MLP
                    zin = [ze, zo]
                    znew = [zpool.tile([128, C], f32r, tag="ze", name="znew_e"),
                            zpool.tile([128, C], f32r, tag="zo", name="znew_o")]
                else:
                    kept, _upd = _layer_masks(li)
                    kz = zo if (li % 2 == 1) else ze
                    uz = ze if (li % 2 == 1) else zo
                    inkts = [kz, pe_t, po_t]
                    w1off, w2soff, w2toff = 0, 0, 2048
                    w3soff, w3toff = 4096, 4608
                    n3 = 1
                    zin = [uz]
                    znew = [zpool.tile([128, C], f32r, tag=("ze" if li % 2 == 1 else "zo"),
                                       name=f"znew_{li}")]
                nk1 = len(inkts)

                for c in range(NCH):
                    cs = slice(c * CH, (c + 1) * CH)
                    h1 = h1pool.tile([128, 8, CH], f32r, tag="h1")
                    for quarter in range(4):
                        ps1 = mmp.tile([128, 2, CH], f32, tag="mm")
                        for m in range(2):
                            mi = quarter * 2 + m
                            for k, kt in enumerate(inkts):
                                nc.tensor.matmul(
                                    ps1[:, m, :],
                                    lhsT=w1t[:, w1off + k * 1024 + mi * 128: w1off + k * 1024 + (mi + 1) * 128],
                                    rhs=kt[:, cs],
                                    start=(k == 0), stop=(k == nk1 - 1))
                            nc.scalar.activation(out=h1[:, mi, :], in_=ps1[:, m, :],
                                                 func=AF.Prelu, bias=bt[:, mi:mi + 1],
                                                 scale=1.0, alpha=0.01)
                    h2 = h2pool.tile([128, 8, CH], f32r, tag="h2")
                    # L2 s (ACT evict) and t (DVE evict)
                    for tgt in range(2):  # 0=s,1=t
                        woff = w2soff if tgt == 0 else w2toff
                        hoff = 0 if tgt == 0 else 4
                        for halfq in range(2):
                            ps2 = mmp.tile([128, 2, CH], f32, tag="mm")
                            for m2 in range(2):
                                m = halfq * 2 + m2
                                for k in range(4):
                                    nc.tensor.matmul(
                                        ps2[:, m2, :],
                                        lhsT=w23[:, woff + k * 512 + m * 128: woff + k * 512 + (m + 1) * 128],
                                        rhs=h1[:, hoff + k, :],
                                        start=(k == 0), stop=(k == 3))
                                bcol = 8 + 4 * tgt + m
                                if tgt == 0:
                                    nc.scalar.activation(out=h2[:, m, :], in_=ps2[:, m2, :],
                                                         func=AF.Prelu, bias=bt[:, bcol:bcol + 1],
                                                         scale=1.0, alpha=0.01)
                                else:
                                    tmp = stpool.tile([128, CH], f32, tag="tb")
                                    nc.vector.tensor_scalar(out=tmp, in0=ps2[:, m2, :],
                                                            scalar1=bt[:, bcol:bcol + 1],
                                                            scalar2=None, op0=ALU.add)
                                    nc.vector.scalar_tensor_tensor(
                                        out=h2[:, 4 + m, :], in0=tmp, scalar=0.01, in1=tmp,
                                        op0=ALU.mult, op1=ALU.max)
                    # L3: s tile (n3 mtiles) and t tile (n3 mtiles)
                    wl3 = w23
                    ps3s = mmp.tile([128, 2, CH], f32, tag="mm")
                    ps3t = mmp.tile([128, 2, CH], f32, tag="mm")
                    for m in range(n3):
                        for k in range(4):
                            nc.tensor.matmul(
                                ps3s[:, m, :],
                                lhsT=wl3[:, w3soff + k * n3 * 128 + m * 128: w3soff + k * n3 * 128 + (m + 1) * 128],
                                rhs=h2[:, k, :], start=(k == 0), stop=(k == 3))
                        for k in range(4):
                            nc.tensor.matmul(
                                ps3t[:, m, :],
                                lhsT=wl3[:, w3toff + k * n3 * 128 + m * 128: w3toff + k * n3 * 128 + (m + 1) * 128],
                                rhs=h2[:, 4 + k, :], start=(k == 0), stop=(k == 3))
                    for m in range(n3):
                        st = stpool.tile([128, CH], f32, tag="s")
                        tt = stpool.tile([128, CH], f32, tag="t")
                        nc.scalar.activation(out=st, in_=ps3s[:, m, :], func=AF.Tanh,
                                             bias=bt[:, 16 + m:17 + m], scale=1.0)
                        nc.scalar.activation(out=tt, in_=ps3t[:, m, :], func=AF.Identity,
                                             bias=bt[:, 18 + m:19 + m], scale=1.0)
                        # ldj accumulation: acc[:,0] += sum(s)
                        nc.vector.tensor_reduce(out=rtmp, in_=st, axis=AX.X, op=ALU.add)
                        nc.vector.tensor_add(acc[:, 0:1], acc[:, 0:1], rtmp)
                        # exp(-s) in place
                        nc.scalar.activation(out=st, in_=st, func=AF.Exp, scale=-1.0)
                        # z update (in-place sub into tt)
                        nc.vector.tensor_sub(tt, zin[m][:, cs], tt)
                        nc.vector.tensor_mul(znew[m][:, cs], tt, st)
                if isc:
                    ze, zo = znew[0], znew[1]
                elif li % 2 == 1:
                    ze = znew[0]
                    if li == 1:
                        zsh_e = _mk_zsh(ze, "zsh_e")
                else:
                    zo = znew[0]
                    if li == 2:
                        zsh_o = _mk_zsh(zo, "zsh_o")

            # z3 outputs + sum(z3^2)
            nc.sync.dma_start(out=d["z3e"][:, :], in_=ze)
            nc.sync.dma_start(out=d["z3o"][:, :], in_=zo)
            zsq = zshp.tile([128, C], f32, tag="zsh_e", name="zsq")
            for zt_ in (ze, zo):
                nc.vector.tensor_mul(zsq, zt_, zt_)
                nc.vector.tensor_reduce(out=rtmp, in_=zsq, axis=AX.X, op=ALU.add)
                nc.vector.tensor_add(acc[:, 1:2], acc[:, 1:2], rtmp)
            zfin[0], zfin[1] = ze, zo

        # ---------------- LSTM prior ----------------
        ze, zo = zfin
        with ExitStack() as ls:
            lp = ls.enter_context(tc.tile_pool(name="lp", bufs=1))

            wlh1 = lp.tile([128, 8192], bf16, tag="wlh1")
            nc.sync.dma_start(out=wlh1, in_=d["wl"][:, OW_HH1:OW_HH1 + 8192])
            wlbh = lp.tile([128, 8192], bf16, tag="wlbh")
            nc.sync.dma_start(out=wlbh, in_=d["wl"][:, OW_HH2:OW_HH2 + 8192])
            wlc = lp.tile([128, 2560], bf16, tag="wlc")
            nc.sync.dma_start(out=wlc[:, 0:2048], in_=d["wl"][:, OW_ML:OW_ML + 2048])
            nc.sync.dma_start(out=wlc[:, 2048:2560], in_=d["wl"][:, OBML:OBML + 512])

            # time-major shifted copies: block 0 = 0, block k = zp_{k-1}
            zt_eo = []
            for src in (ze, zo):
                zt_ = lp.tile([128, (T + 1) * BL], bf16, tag=f"zt{len(zt_eo)}",
                              name=f"zt{len(zt_eo)}")
                z3v = zt_.rearrange("p (t s) -> p t s", s=BL)
                nc.vector.memset(z3v[:, 0:1, :], 0.0)
                nc.vector.tensor_copy(out=z3v[:, 1:T + 1, :],
                                      in_=src.rearrange("p (s t) -> p t s", t=T))
                zt_eo.append(z3v)
            h2all = lp.tile([128, 4, T, BL], bf16, tag="h2all")

            with ExitStack() as rs:
                gps = rs.enter_context(tc.tile_pool(name="gps", bufs=2, space="PSUM"))
                rot = rs.enter_context(tc.tile_pool(name="rot", bufs=2))
                h1c0a = rot.tile([128, 2, BL], bf16, tag="h1ca")
                h1c0b = rot.tile([128, 2, BL], bf16, tag="h1cb")
                h1c = (h1c0a, h1c0b)
                c1c = rot.tile([128, 4, BL], f32, tag="c1_init")
                c2c = rot.tile([128, 4, BL], f32, tag="c2_init")
                h20 = lp.tile([128, 4, BL], bf16, tag="h20")
                nc.vector.memset(h1c0a, 0.0)
                nc.vector.memset(h1c0b, 0.0)
                nc.vector.memset(c1c, 0.0)
                nc.vector.memset(c2c, 0.0)
                nc.vector.memset(h20, 0.0)

                def cell(pg, wtx, wth, wih_off, whh_off, b_off, nktx, nkth, xk, hk):
                    for mt in range(16):
                        only_b = (nktx == 0 and nkth == 0)
                        nc.tensor.matmul(pg[:, mt, :],
                                         lhsT=wtx[:, b_off + mt * 128: b_off + (mt + 1) * 128],
                                         rhs=ones[:, 0:BL], start=True, stop=only_b)
                        for k in range(nktx):
                            nc.tensor.matmul(pg[:, mt, :],
                                             lhsT=wtx[:, wih_off + k * 2048 + mt * 128: wih_off + k * 2048 + (mt + 1) * 128],
                                             rhs=xk(k), start=False,
                                             stop=(nkth == 0 and k == nktx - 1))
                        for k in range(nkth):
                            nc.tensor.matmul(pg[:, mt, :],
                                             lhsT=wth[:, whh_off + k * 2048 + mt * 128: whh_off + k * 2048 + (mt + 1) * 128],
                                             rhs=hk(k), start=False, stop=(k == nkth - 1))

                def chain(pg, cprev, hout_fn, tagp):
                    # gate layout (host-permuted): [i0 i1 f0 f1 | i2 i3 f2 f3 | g | o]
                    cn = rot.tile([128, 4, BL], f32, tag=f"c{tagp[0]}c", name=f"cn_{tagp}")
                    for hf in range(2):
                        sl4 = slice(hf * 4, hf * 4 + 4)
                        gifh = rot.tile([128, 4, BL], f32, tag="gif", name=f"gif_{tagp}{hf}")
                        ggh = rot.tile([128, 2, BL], f32, tag="gg", name=f"gg_{tagp}{hf}")
                        goh = rot.tile([128, 2, BL], f32, tag="go", name=f"go_{tagp}{hf}")
                        nc.scalar.activation(out=gifh, in_=pg[:, sl4, :], func=AF.Sigmoid)
                        nc.scalar.activation(out=ggh, in_=pg[:, 8 + 2 * hf:10 + 2 * hf, :], func=AF.Tanh)
                        nc.scalar.activation(out=goh, in_=pg[:, 12 + 2 * hf:14 + 2 * hf, :], func=AF.Sigmoid)
                        s2 = slice(2 * hf, 2 * hf + 2)
                        tm1 = rot.tile([128, 2, BL], f32, tag="tm1", name=f"tm1_{tagp}{hf}")
                        tm2 = rot.tile([128, 2, BL], f32, tag="tm2", name=f"tm2_{tagp}{hf}")
                        nc.vector.tensor_mul(tm1, gifh[:, 2:4, :], cprev[:, s2, :])
                        nc.vector.tensor_mul(tm2, gifh[:, 0:2, :], ggh)
                        nc.vector.tensor_add(cn[:, s2, :], tm1, tm2)
                        tc_ = rot.tile([128, 2, BL], f32, tag="tc1", name=f"tc_{tagp}{hf}")
                        nc.scalar.activation(out=tc_, in_=cn[:, s2, :], func=AF.Tanh)
                        nc.vector.tensor_mul(hout_fn(hf), goh, tc_)
                    return cn

                for t in range(T):
                    pg1 = gps.tile([128, 16, BL], f32, tag="pg", name=f"pg1_{t}")
                    cell(pg1, wla, wlh1, 0, 0, 4096,
                         2 if t > 0 else 0, 4 if t > 0 else 0,
                         lambda k, t=t: zt_eo[k][:, t, :],
                         lambda k, h=h1c: h[k // 2][:, k % 2, :])
                    h1na = rot.tile([128, 2, BL], bf16, tag="h1ca", name=f"h1na_{t}")
                    h1nb = rot.tile([128, 2, BL], bf16, tag="h1cb", name=f"h1nb_{t}")
                    h1n = (h1na, h1nb)
                    c1c = chain(pg1, c1c, lambda hf, a=h1na, b=h1nb: (a if hf == 0 else b),
                                f"1_{t}")

                    pg2 = gps.tile([128, 16, BL], f32, tag="pg", name=f"pg2_{t}")
                    cell(pg2, wlbx, wlbh, 0, 0, 8192,
                         4, 4 if t > 0 else 0,
                         lambda k, h=h1n: h[k // 2][:, k % 2, :],
                         lambda k, t=t: h2all[:, k, t - 1, :] if t > 0 else h20[:, k, :])
                    c2c = chain(pg2, c2c,
                                lambda hf, t=t: h2all[:, 2 * hf:2 * hf + 2, t, :], f"2_{t}")
                    h1c = h1n

            # mean/logvar batched over all t
            with ExitStack() as ms:
                mps = ms.enter_context(tc.tile_pool(name="mps", bufs=2, space="PSUM"))
                mlp = ms.enter_context(tc.tile_pool(name="mlp", bufs=1))
                for c4 in range(4):
                    pml = mps.tile([128, 4, CH], f32, tag="pml")
                    for mt in range(4):
                        nc.tensor.matmul(pml[:, mt, :],
                                         lhsT=wlc[:, 2048 + mt * 128: 2048 + (mt + 1) * 128],
                                         rhs=ones[:, :], start=True, stop=False)
                        for k in range(4):
                            nc.tensor.matmul(
                                pml[:, mt, :],
                                lhsT=wlc[:, k * 512 + mt * 128: k * 512 + (mt + 1) * 128],
                                rhs=h2all[:, k, 4 * c4:4 * c4 + 4, :].rearrange("p a b -> p (a b)"),
                                start=False, stop=(k == 3))
                    mean = mlp.tile([128, 2, CH], f32, tag="mean")
                    elv = mlp.tile([128, 2, CH], f32, tag="elv")
                    nc.scalar.activation(out=mean, in_=pml[:, 0:2, :], func=AF.Copy)
                    nc.scalar.activation(out=elv, in_=pml[:, 2:4, :], func=AF.Exp, scale=-1.0)
                    # sum(logvar)
                    nc.vector.tensor_reduce(out=rtmp, in_=pml[:, 2:4, :], axis=AX.XY, op=ALU.add)
                    nc.vector.tensor_add(acc[:, 3:4], acc[:, 3:4], rtmp)
                    # samp = (zp_t - mean) * elv ; acc2 += sum(samp^2)
                    for hf in range(2):
                        zpv = zt_eo[hf][:, 4 * c4 + 1:4 * c4 + 5, :].rearrange("p a b -> p (a b)")
                        sd = mlp.tile([128, CH], f32, tag="sd")
                        nc.vector.tensor_sub(sd, zpv, mean[:, hf, :])
                        sm = mlp.tile([128, CH], f32, tag="sm")
                        nc.vector.tensor_mul(sm, sd, elv[:, hf, :])
                        nc.vector.tensor_mul(sm, sm, sm)
                        nc.vector.tensor_reduce(out=rtmp, in_=sm, axis=AX.X, op=ALU.add)
                        nc.vector.tensor_add(acc[:, 2:3], acc[:, 2:3], rtmp)

        nc.sync.dma_start(out=d["accs"][:, :], in_=acc)
    nc.finalize()
    return nc


def _prep_inmaps(inputs):
    g = {k: np.asarray(v, dtype=np.float32) for k, v in inputs.items()}
    nf = g["nf_input"].reshape(B, T, 2 * Z)

    # flow weight blob
    wf = np.zeros((128, 10 * WAB_COLS), np.float32)
    bf = np.zeros((128, NL * 20), np.float32)
    for j, li in enumerate([8, 7, 6, 5, 4, 3, 2, 1, 0]):
        Wcat = np.concatenate([g["sW1"][li], g["tW1"][li]], 0)  # [1024, 512]
        if li == 0:
            kts = [Wcat[:, 0:256:2], Wcat[:, 1:256:2], Wcat[:, 256::2], Wcat[:, 257::2]]
            w1off, w2soff, w2toff = 0, 4096, 6144
        else:
            kept, _ = _layer_masks(li)
            kts = [Wcat[:, kept], Wcat[:, 256 + EV], Wcat[:, 256 + OD]]
            w1off, w2soff, w2toff = 0, 3072, 5120
        base = j * WAB_COLS
        for k, kk in enumerate(kts):
            wf[:, base + w1off + k * 1024: base + w1off + (k + 1) * 1024] = kk.T
        for k in range(4):
            wf[:, base + w2soff + k * 512: base + w2soff + (k + 1) * 512] = \
                g["sW2"][li].T[k * 128:(k + 1) * 128, :]
            wf[:, base + w2toff + k * 512: base + w2toff + (k + 1) * 512] = \
                g["tW2"][li].T[k * 128:(k + 1) * 128, :]
        if li == 0:
            W3s = g["sW3"][0][PERM_EO, :]   # [256, 512]
            W3t = g["tW3"][0][PERM_EO, :]
            b3s = g["sb3"][0][PERM_EO]
            b3t = g["tb3"][0][PERM_EO]
            base2 = 9 * WAB_COLS
            for k in range(4):
                wf[:, base2 + k * 256: base2 + (k + 1) * 256] = W3s.T[k * 128:(k + 1) * 128, :]
                wf[:, base2 + 1024 + k * 256: base2 + 1024 + (k + 1) * 256] = \
                    W3t.T[k * 128:(k + 1) * 128, :]
            bf[:, j * 20 + 16] = b3s[0:128]
            bf[:, j * 20 + 17] = b3s[128:256]
            bf[:, j * 20 + 18] = b3t[0:128]
            bf[:, j * 20 + 19] = b3t[128:256]
        else:
            _, upd = _layer_masks(li)
            W3s = g["sW3"][li][upd, :]      # [128, 512]
            W3t = g["tW3"][li][upd, :]
            for k in range(4):
                wf[:, base + 7168 + k * 128: base + 7168 + (k + 1) * 128] = \
                    W3s.T[k * 128:(k + 1) * 128, :]
                wf[:, base + 7680 + k * 128: base + 7680 + (k + 1) * 128] = \
                    W3t.T[k * 128:(k + 1) * 128, :]
            bf[:, j * 20 + 16] = g["sb3"][li][upd]
            bf[:, j * 20 + 18] = g["tb3"][li][upd]
        b1cat = np.concatenate([g["sb1"][li], g["tb1"][li]])
        bf[:, j * 20:j * 20 + 8] = b1cat.reshape(8, 128).T
        bf[:, j * 20 + 8:j * 20 + 12] = g["sb2"][li].reshape(4, 128).T
        bf[:, j * 20 + 12:j * 20 + 16] = g["tb2"][li].reshape(4, 128).T

    # LSTM blob
    wlb = np.zeros((128, WL_COLS), np.float32)

    def put_kt(off, mat_t, width):
        # mat_t: [K, width] -> kt-major tiles [128, width]
        nkt = mat_t.shape[0] // 128
        for k in range(nkt):
            wlb[:, off + k * width: off + (k + 1) * width] = mat_t[k * 128:(k + 1) * 128, :]

    mtp = np.concatenate([np.arange(b * 128, (b + 1) * 128)
                          for b in (0, 1, 4, 5, 2, 3, 6, 7, 8, 9, 10, 11, 12, 13, 14, 15)])
    put_kt(OW_IH1, g["l1Wih"][mtp, :].T[PERM_EO, :], 2048)
    put_kt(OW_HH1, g["l1Whh"][mtp, :].T, 2048)
    put_kt(OW_IH2, g["l2Wih"][mtp, :].T, 2048)
    put_kt(OW_HH2, g["l2Whh"][mtp, :].T, 2048)
    Wml = np.concatenate([g["mW"][PERM_EO, :], g["lvW"][PERM_EO, :]], 0)  # [512, 512]
    put_kt(OW_ML, Wml.T, 512)
    wlb[:, OB1:OB1 + 2048] = np.broadcast_to(
        (g["l1bih"] + g["l1bhh"])[mtp][None, :] / 128.0, (128, 2048))
    wlb[:, OB2:OB2 + 2048] = np.broadcast_to(
        (g["l2bih"] + g["l2bhh"])[mtp][None, :] / 128.0, (128, 2048))
    bml = np.concatenate([g["mb"][PERM_EO], g["lvb"][PERM_EO]])
    wlb[:, OBML:OBML + 512] = np.broadcast_to(bml[None, :] / 128.0, (128, 512))

    import ml_dtypes
    wlb = wlb.astype(ml_dtypes.bfloat16)

    in_maps = []
    for cidx in range(NCORES):
        nfc = nf[cidx * BL:(cidx + 1) * BL]              # [BL, T, 512]
        x = nfc[:, :, Z:]
        p = nfc[:, :, :Z]
        # feature-major [128, C]: partition = channel pair index, col = s*16+t
        m = {
            "xe": np.ascontiguousarray(x[:, :, 0::2].transpose(2, 0, 1).reshape(128, C)),
            "xo": np.ascontiguousarray(x[:, :, 1::2].transpose(2, 0, 1).reshape(128, C)),
            "pe": np.ascontiguousarray(p[:, :, 0::2].transpose(2, 0, 1).reshape(128, C)),
            "po": np.ascontiguousarray(p[:, :, 1::2].transpose(2, 0, 1).reshape(128, C)),
            "wf": wf, "bf": bf, "wl": wlb,
        }
        in_maps.append(m)
    return in_maps


def kernel(**inputs):
    if "nc" not in _CACHE:
        _CACHE["nc"] = _build_nc()
    nc = _CACHE["nc"]
    in_maps = _prep_inmaps(inputs)
    res_obj = run_bass_kernel_spmd(nc, in_maps, list(range(NCORES)),
                                   trace=bool(os.environ.get("KERNEL_TRACE")))
    _CACHE["last"] = res_obj
    res = res_obj.results

    z3 = np.empty((B, T, Z), np.float32)
    s_s = s_z2 = s_sp2 = s_lv = 0.0
    for cidx in range(NCORES):
        r = res[cidx]
        ze = r["z3e"].reshape(128, BL, T)
        zo = r["z3o"].reshape(128, BL, T)
        z3[cidx * BL:(cidx + 1) * BL, :, 0::2] = ze.transpose(1, 2, 0)
        z3[cidx * BL:(cidx + 1) * BL, :, 1::2] = zo.transpose(1, 2, 0)
        a = r["accs"].astype(np.float64)
        s_s += a[:, 0].sum()
        s_z2 += a[:, 1].sum()
        s_sp2 += a[:, 2].sum()
        s_lv += a[:, 3].sum()

    n_el = float(B) * T * Z
    sum_ldj = -s_s
    sum_logN_z3 = -0.5 * s_z2 - n_el * LOG_SQRT_2PI
    sum_logN_samp = -0.5 * s_sp2 - n_el * LOG_SQRT_2PI
    sum_logp2 = -s_lv
    loglik = (sum_logN_samp + sum_logp2) * 0.0025 + (sum_logN_z3 + sum_ldj)
    loss_q = np.float32(-loglik / B)
    return loss_q, z3


# revision 30
# speedup vs baseline: 1.0117x; 1.0117x over previous
"""Trainium2 Bass kernel for nn_DisentangledVAE (9-layer coupling flow + 2-layer LSTM prior).

Self-contained: builds per-core feature-major layout on host, runs SPMD on 8 NeuronCores.
Flow matmuls in float32r (full PE rate at N=512, ~1.4e-4 err), LSTM in bf16 (loss-weight 0.0025).
"""
import os
import sys

sys.path.insert(0, '/opt/trn_rl_repo')
import numpy as np
import concourse.bass as bass  # noqa: F401
from concourse.bacc import Bacc
import concourse.tile as tile
from concourse import mybir
from concourse.bass_utils import run_bass_kernel_spmd

f32 = mybir.dt.float32
f32r = mybir.dt.float32r
bf16 = mybir.dt.bfloat16
AF = mybir.ActivationFunctionType
ALU = mybir.AluOpType
AX = mybir.AxisListType

Z = 256
T = 16
NF = 512
H = 512
NL = 9
B = 1024
NCORES = 8
BL = B // NCORES          # 128 sequences per core
C = BL * T                # 2048 columns per core (col = s*16 + t)
CH = 512                  # N-chunk
NCH = C // CH
LOG_SQRT_2PI = 0.9189385332046727

EV = np.arange(0, Z, 2)
OD = np.arange(1, Z, 2)
PERM_EO = np.concatenate([EV, OD])

# flow weight blob (f32r): 8 a/b layers (8192 cols each) + 'c' piece1 (8192) + 'c' piece2 (2048)
WAB_COLS = 8192
# LSTM blob (bf16) col offsets
OW_IH1, OW_HH1, OW_IH2, OW_HH2, OW_ML, OB1, OB2, OBML = (
    0, 4096, 12288, 20480, 28672, 30720, 32768, 34816)
WL_COLS = 35328

_CACHE = {}


def _layer_masks(i):
    # i odd -> mask 'a' (odd channels kept); i even>=2 -> 'b' (even kept); i==0 -> 'c'
    if i == 0:
        return None, None
    if i % 2 == 1:
        return OD, EV   # kept, updated
    return EV, OD


def _build_nc():
    nc = Bacc()
    d = {}
    for nm in ("xe", "xo", "pe", "po"):
        d[nm] = nc.declare_dram_parameter(nm, [128, C], f32r, isOutput=False)
    d["wf"] = nc.declare_dram_parameter("wf", [128, 10 * WAB_COLS], f32r, isOutput=False)
    d["bf"] = nc.declare_dram_parameter("bf", [128, NL * 20], f32, isOutput=False)
    d["wl"] = nc.declare_dram_parameter("wl", [128, WL_COLS], bf16, isOutput=False)
    d["z3e"] = nc.declare_dram_parameter("z3e", [128, C], f32r, isOutput=True)
    d["z3o"] = nc.declare_dram_parameter("z3o", [128, C], f32r, isOutput=True)
    d["accs"] = nc.declare_dram_parameter("accs", [128, 4], f32, isOutput=True)

    from contextlib import ExitStack
    with tile.TileContext(nc) as tc, ExitStack() as top:
        pers = top.enter_context(tc.tile_pool(name="pers", bufs=1))
        zpool = top.enter_context(tc.tile_pool(name="zp", bufs=2))

        pe_t = pers.tile([128, C], f32r, tag="pe")
        po_t = pers.tile([128, C], f32r, tag="po")
        acc = pers.tile([128, 4], f32, tag="acc")
        nc.vector.memset(acc, 0.0)
        rtmp = pers.tile([128, 1], f32, tag="rtmp")
        zfin = [None, None]

        ze = zpool.tile([128, C], f32r, tag="ze")
        zo = zpool.tile([128, C], f32r, tag="zo")
        nc.sync.dma_start(out=ze, in_=d["xe"][:, :])
        nc.sync.dma_start(out=pe_t, in_=d["pe"][:, :])
        nc.sync.dma_start(out=po_t, in_=d["po"][:, :])

        # cell1 x-weights+bias: no-dependency DMA at kernel start, hidden under flow
        wlp = top.enter_context(tc.tile_pool(name="wlp", bufs=1))
        wla = wlp.tile([128, 6144], bf16, tag="wla")
        wlbx = wlp.tile([128, 10240], bf16, tag="wlbx")
        ones = wlp.tile([128, CH], bf16, tag="ones")
        nc.vector.memset(ones, 1.0)

        # ---------------- flow ----------------
        with ExitStack() as fs:
            wpool = fs.enter_context(tc.tile_pool(name="wp", bufs=2))
            bpool = fs.enter_context(tc.tile_pool(name="bp", bufs=2))
            h1pool = fs.enter_context(tc.tile_pool(name="h1p", bufs=1))
            h2pool = fs.enter_context(tc.tile_pool(name="h2p", bufs=1))
            stpool = fs.enter_context(tc.tile_pool(name="stp", bufs=2))
            mmp = fs.enter_context(tc.tile_pool(name="mmp", bufs=4, space="PSUM"))
            zshp = fs.enter_context(tc.tile_pool(name="zshp", bufs=1))

            def _mk_zsh(srct, name):
                dst = zshp.tile([128, C], f32r, tag=name, name=name)
                s3 = srct.rearrange("p (s t) -> p s t", t=T)
                d3 = dst.rearrange("p (s t) -> p s t", t=T)
                nc.vector.tensor_copy(out=d3[:, :, 1:T], in_=s3[:, :, 0:T - 1])
                nc.vector.memset(d3[:, :, 0:1].bitcast(f32), 0.0)
                return dst

            for j, li in enumerate([8, 7, 6, 5, 4, 3, 2, 1, 0]):
                isc = (li == 0)
                base = j * WAB_COLS
                w1t = wpool.tile([128, 4096], f32r, tag="w1", bufs=1)
                w23 = wpool.tile([128, 6144], f32r, tag="w23")
                if isc:
                    nc.sync.dma_start(out=w1t[:, 0:4096], in_=d["wf"][:, base:base + 4096])
                    nc.sync.dma_start(out=w23[:, 0:4096], in_=d["wf"][:, base + 4096:base + 8192])
                    nc.sync.dma_start(out=w23[:, 4096:6144],
                                      in_=d["wf"][:, 9 * WAB_COLS:9 * WAB_COLS + 2048])
                else:
                    nc.sync.dma_start(out=w1t[:, 0:3072], in_=d["wf"][:, base:base + 3072])
                bt = bpool.tile([128, 20], f32, tag="b")
                nc.sync.dma_start(out=bt, in_=d["bf"][:, j * 20:(j + 1) * 20])
                if not isc:
                    nc.sync.dma_start(out=w23[:, 0:5120], in_=d["wf"][:, base + 3072:base + 8192])
                if j == 0:
                    nc.sync.dma_start(out=zo, in_=d["xo"][:, :])
                    nc.sync.dma_start(out=wla[:, 0:4096], in_=d["wl"][:, 0:4096])
                    nc.sync.dma_start(out=wla[:, 4096:6144], in_=d["wl"][:, OB1:OB1 + 2048])
                    nc.sync.dma_start(out=wlbx[:, 0:8192], in_=d["wl"][:, OW_IH2:OW_IH2 + 8192])
                    nc.sync.dma_start(out=wlbx[:, 8192:10240], in_=d["wl"][:, OB2:OB2 + 2048])

                if isc:
                    inkts = [zsh_e, zsh_o, pe_t, po_t]
                    w1off, w2soff, w2toff = 0, 0, 2048
                    w3soff, w3toff = 4096, 5120
                    n3 = 2  # L3 mtiles per MLP
                    zin = [ze, zo]
                    znew = [zpool.tile([128, C], f32r, tag="ze", name="znew_e"),
                            zpool.tile([128, C], f32r, tag="zo", name="znew_o")]
                else:
                    kept, _upd = _layer_masks(li)
                    kz = zo if (li % 2 == 1) else ze
                    uz = ze if (li % 2 == 1) else zo
                    inkts = [kz, pe_t, po_t]
                    w1off, w2soff, w2toff = 0, 0, 2048
                    w3soff, w3toff = 4096, 4608
                    n3 = 1
                    zin = [uz]
                    znew = [zpool.tile([128, C], f32r, tag=("ze" if li % 2 == 1 else "zo"),
                                       name=f"znew_{li}")]
                nk1 = len(inkts)

                for c in range(NCH):
                    cs = slice(c * CH, (c + 1) * CH)
                    h1 = h1pool.tile([128, 8, CH], f32r, tag="h1")
                    for quarter in range(4):
                        ps1 = mmp.tile([128, 2, CH], f32, tag="mm")
                        for m in range(2):
                            mi = quarter * 2 + m
                            for k, kt in enumerate(inkts):
                                nc.tensor.matmul(
                                    ps1[:, m, :],
                                    lhsT=w1t[:, w1off + k * 1024 + mi * 128: w1off + k * 1024 + (mi + 1) * 128],
                                    rhs=kt[:, cs],
                                    start=(k == 0), stop=(k == nk1 - 1))
                            nc.scalar.activation(out=h1[:, mi, :], in_=ps1[:, m, :],
                                                 func=AF.Prelu, bias=bt[:, mi:mi + 1],
                                                 scale=1.0, alpha=0.01)
                    h2 = h2pool.tile([128, 8, CH], f32r, tag="h2")
                    # L2 s (ACT evict) and t (DVE evict)
                    for tgt in range(2):  # 0=s,1=t
                        woff = w2soff if tgt == 0 else w2toff
                        hoff = 0 if tgt == 0 else 4
                        for halfq in range(2):
                            ps2 = mmp.tile([128, 2, CH], f32, tag="mm")
                            for m2 in range(2):
                                m = halfq * 2 + m2
                                for k in range(4):
                                    nc.tensor.matmul(
                                        ps2[:, m2, :],
                                        lhsT=w23[:, woff + k * 512 + m * 128: woff + k * 512 + (m + 1) * 128],
                                        rhs=h1[:, hoff + k, :],
                                        start=(k == 0), stop=(k == 3))
                                bcol = 8 + 4 * tgt + m
                                if tgt == 0:
                                    nc.scalar.activation(out=h2[:, m, :], in_=ps2[:, m2, :],
                                                         func=AF.Prelu, bias=bt[:, bcol:bcol + 1],
                                                         scale=1.0, alpha=0.01)
                                else:
                                    tmp = stpool.tile([128, CH], f32, tag="tb")
                                    nc.vector.tensor_scalar(out=tmp, in0=ps2[:, m2, :],
                                                            scalar1=bt[:, bcol:bcol + 1],
                                                            scalar2=None, op0=ALU.add)
                                    nc.vector.scalar_tensor_tensor(
                                        out=h2[:, 4 + m, :], in0=tmp, scalar=0.01, in1=tmp,
                                        op0=ALU.mult, op1=ALU.max)
                    # L3: s tile (n3 mtiles) and t tile (n3 mtiles)
                    wl3 = w23
                    ps3s = mmp.tile([128, 2, CH], f32, tag="mm")
                    ps3t = mmp.tile([128, 2, CH], f32, tag="mm")
                    for m in range(n3):
                        for k in range(4):
                            nc.tensor.matmul(
                                ps3s[:, m, :],
                                lhsT=wl3[:, w3soff + k * n3 * 128 + m * 128: w3soff + k * n3 * 128 + (m + 1) * 128],
                                rhs=h2[:, k, :], start=(k == 0), stop=(k == 3))
                        for k in range(4):
                            nc.tensor.matmul(
                                ps3t[:, m, :],
                                lhsT=wl3[:, w3toff + k * n3 * 128 + m * 128: w3toff + k * n3 * 128 + (m + 1) * 128],
                                rhs=h2[:, 4 + k, :], start=(k == 0), stop=(k == 3))
                    for m in range(n3):
                        st = stpool.tile([128, CH], f32, tag="s")
                        tt = stpool.tile([128, CH], f32, tag="t")
                        nc.scalar.activation(out=st, in_=ps3s[:, m, :], func=AF.Tanh,
                                             bias=bt[:, 16 + m:17 + m], scale=1.0)
                        nc.scalar.activation(out=tt, in_=ps3t[:, m, :], func=AF.Identity,
                                             bias=bt[:, 18 + m:19 + m], scale=1.0)
                        # ldj accumulation: acc[:,0] += sum(s)
                        nc.vector.tensor_reduce(out=rtmp, in_=st, axis=AX.X, op=ALU.add)
                        nc.vector.tensor_add(acc[:, 0:1], acc[:, 0:1], rtmp)
                        # exp(-s) in place
                        nc.scalar.activation(out=st, in_=st, func=AF.Exp, scale=-1.0)
                        # z update (in-place sub into tt)
                        nc.vector.tensor_sub(tt, zin[m][:, cs], tt)
                        nc.vector.tensor_mul(znew[m][:, cs], tt, st)
                if isc:
                    ze, zo = znew[0], znew[1]
                elif li % 2 == 1:
                    ze = znew[0]
                    if li == 1:
                        zsh_e = _mk_zsh(ze, "zsh_e")
                else:
                    zo = znew[0]
                    if li == 2:
                        zsh_o = _mk_zsh(zo, "zsh_o")

            # z3 outputs + sum(z3^2)
            nc.sync.dma_start(out=d["z3e"][:, :], in_=ze)
            nc.sync.dma_start(out=d["z3o"][:, :], in_=zo)
            zsq = zshp.tile([128, C], f32, tag="zsh_e", name="zsq")
            for zt_ in (ze, zo):
                nc.vector.tensor_mul(zsq, zt_, zt_)
                nc.vector.tensor_reduce(out=rtmp, in_=zsq, axis=AX.X, op=ALU.add)
                nc.vector.tensor_add(acc[:, 1:2], acc[:, 1:2], rtmp)
            zfin[0], zfin[1] = ze, zo

        # ---------------- LSTM prior ----------------
        ze, zo = zfin
        with ExitStack() as ls:
            lp = ls.enter_context(tc.tile_pool(name="lp", bufs=1))

            wlh1 = lp.tile([128, 8192], bf16, tag="wlh1")
            nc.sync.dma_start(out=wlh1, in_=d["wl"][:, OW_HH1:OW_HH1 + 8192])
            wlbh = lp.tile([128, 8192], bf16, tag="wlbh")
            nc.sync.dma_start(out=wlbh, in_=d["wl"][:, OW_HH2:OW_HH2 + 8192])
            wlc = lp.tile([128, 2560], bf16, tag="wlc")
            nc.sync.dma_start(out=wlc[:, 0:2048], in_=d["wl"][:, OW_ML:OW_ML + 2048])
            nc.sync.dma_start(out=wlc[:, 2048:2560], in_=d["wl"][:, OBML:OBML + 512])

            # time-major shifted copies: block 0 = 0, block k = zp_{k-1}
            zt_eo = []
            for src in (ze, zo):
                zt_ = lp.tile([128, (T + 1) * BL], bf16, tag=f"zt{len(zt_eo)}",
                              name=f"zt{len(zt_eo)}")
                z3v = zt_.rearrange("p (t s) -> p t s", s=BL)
                nc.vector.memset(z3v[:, 0:1, :], 0.0)
                nc.vector.tensor_copy(out=z3v[:, 1:T + 1, :],
                                      in_=src.rearrange("p (s t) -> p t s", t=T))
                zt_eo.append(z3v)
            h2all = lp.tile([128, 4, T, BL], bf16, tag="h2all")

            with ExitStack() as rs:
                gps = rs.enter_context(tc.tile_pool(name="gps", bufs=2, space="PSUM"))
                rot = rs.enter_context(tc.tile_pool(name="rot", bufs=2))
                h1c0a = rot.tile([128, 2, BL], bf16, tag="h1ca")
                h1c0b = rot.tile([128, 2, BL], bf16, tag="h1cb")
                h1c = (h1c0a, h1c0b)
                c1c = rot.tile([128, 4, BL], f32, tag="c1_init")
                c2c = rot.tile([128, 4, BL], f32, tag="c2_init")
                h20 = lp.tile([128, 4, BL], bf16, tag="h20")
                nc.vector.memset(h1c0a, 0.0)
                nc.vector.memset(h1c0b, 0.0)
                nc.vector.memset(c1c, 0.0)
                nc.vector.memset(c2c, 0.0)
                nc.vector.memset(h20, 0.0)

                def cell(pg, wtx, wth, wih_off, whh_off, b_off, nktx, nkth, xk, hk):
                    for mt in range(16):
                        only_b = (nktx == 0 and nkth == 0)
                        nc.tensor.matmul(pg[:, mt, :],
                                         lhsT=wtx[:, b_off + mt * 128: b_off + (mt + 1) * 128],
                                         rhs=ones[:, 0:BL], start=True, stop=only_b)
                        for k in range(nktx):
                            nc.tensor.matmul(pg[:, mt, :],
                                             lhsT=wtx[:, wih_off + k * 2048 + mt * 128: wih_off + k * 2048 + (mt + 1) * 128],
                                             rhs=xk(k), start=False,
                                             stop=(nkth == 0 and k == nktx - 1))
                        for k in range(nkth):
                            nc.tensor.matmul(pg[:, mt, :],
                                             lhsT=wth[:, whh_off + k * 2048 + mt * 128: whh_off + k * 2048 + (mt + 1) * 128],
                                             rhs=hk(k), start=False, stop=(k == nkth - 1))

                def chain(pg, cprev, hout_fn, tagp):
                    # gate layout (host-permuted): [i0 i1 f0 f1 | i2 i3 f2 f3 | g | o]
                    cn = rot.tile([128, 4, BL], f32, tag=f"c{tagp[0]}c", name=f"cn_{tagp}")
                    for hf in range(2):
                        sl4 = slice(hf * 4, hf * 4 + 4)
                        gifh = rot.tile([128, 4, BL], f32, tag="gif", name=f"gif_{tagp}{hf}")
                        ggh = rot.tile([128, 2, BL], f32, tag="gg", name=f"gg_{tagp}{hf}")
                        goh = rot.tile([128, 2, BL], f32, tag="go", name=f"go_{tagp}{hf}")
                        nc.scalar.activation(out=gifh, in_=pg[:, sl4, :], func=AF.Sigmoid)
                        nc.scalar.activation(out=ggh, in_=pg[:, 8 + 2 * hf:10 + 2 * hf, :], func=AF.Tanh)
                        nc.scalar.activation(out=goh, in_=pg[:, 12 + 2 * hf:14 + 2 * hf, :], func=AF.Sigmoid)
                        s2 = slice(2 * hf, 2 * hf + 2)
                        tm1 = rot.tile([128, 2, BL], f32, tag="tm1", name=f"tm1_{tagp}{hf}")
                        tm2 = rot.tile([128, 2, BL], f32, tag="tm2", name=f"tm2_{tagp}{hf}")
                        nc.vector.tensor_mul(tm1, gifh[:, 2:4, :], cprev[:, s2, :])
                        nc.vector.tensor_mul(tm2, gifh[:, 0:2, :], ggh)
                        nc.vector.tensor_add(cn[:, s2, :], tm1, tm2)
                        tc_ = rot.tile([128, 2, BL], f32, tag="tc1", name=f"tc_{tagp}{hf}")
                        nc.scalar.activation(out=tc_, in_=cn[:, s2, :], func=AF.Tanh)
                        nc.vector.tensor_mul(hout_fn(hf), goh, tc_)
                    return cn

                for t in range(T):
                    pg1 = gps.tile([128, 16, BL], f32, tag="pg", name=f"pg1_{t}")
                    cell(pg1, wla, wlh1, 0, 0, 4096,
                         2 if t > 0 else 0, 4 if t > 0 else 0,
                         lambda k, t=t: zt_eo[k][:, t, :],
                         lambda k, h=h1c: h[k // 2][:, k % 2, :])
                    h1na = rot.tile([128, 2, BL], bf16, tag="h1ca", name=f"h1na_{t}")
                    h1nb = rot.tile([128, 2, BL], bf16, tag="h1cb", name=f"h1nb_{t}")
                    h1n = (h1na, h1nb)
                    c1c = chain(pg1, c1c, lambda hf, a=h1na, b=h1nb: (a if hf == 0 else b),
                                f"1_{t}")

                    pg2 = gps.tile([128, 16, BL], f32, tag="pg", name=f"pg2_{t}")
                    cell(pg2, wlbx, wlbh, 0, 0, 8192,
                         4, 4 if t > 0 else 0,
                         lambda k, h=h1n: h[k // 2][:, k % 2, :],
                         lambda k, t=t: h2all[:, k, t - 1, :] if t > 0 else h20[:, k, :])
                    c2c = chain(pg2, c2c,
                                lambda hf, t=t: h2all[:, 2 * hf:2 * hf + 2, t, :], f"2_{t}")
                    h1c = h1n

            # mean/logvar batched over all t
            with ExitStack() as ms:
                mps = ms.enter_context(tc.tile_pool(name="mps", bufs=2, space="PSUM"))
                mlp = ms.enter_context(tc.tile_pool(name="mlp", bufs=1))
                for c4 in range(4):
                    pml = mps.tile([128, 4, CH], f32, tag="pml")
                    for mt in range(4):
                        nc.tensor.matmul(pml[:, mt, :],
                                         lhsT=wlc[:, 2048 + mt * 128: 2048 + (mt + 1) * 128],
                                         rhs=ones[:, :], start=True, stop=False)
                        for k in range(4):
                            nc.tensor.matmul(
                                pml[:, mt, :],
                                lhsT=wlc[:, k * 512 + mt * 128: k * 512 + (mt + 1) * 128],
                                rhs=h2all[:, k, 4 * c4:4 * c4 + 4, :].rearrange("p a b -> p (a b)"),
                                start=False, stop=(k == 3))
                    mean = mlp.tile([128, 2, CH], f32, tag="mean")
                    elv = mlp.tile([128, 2, CH], f32, tag="elv")
                    nc.scalar.activation(out=mean, in_=pml[:, 0:2, :], func=AF.Copy)
                    nc.scalar.activation(out=elv, in_=pml[:, 2:4, :], func=AF.Exp, scale=-1.0)
                    # sum(logvar)
                    nc.vector.tensor_reduce(out=rtmp, in_=pml[:, 2:4, :], axis=AX.XY, op=ALU.add)
                    nc.vector.tensor_add(acc[:, 3:4], acc[:, 3:4], rtmp)
                    # samp = (zp_t - mean) * elv ; acc2 += sum(samp^2)
                    for hf in range(2):
                        zpv = zt_eo[hf][:, 4 * c4 + 1:4 * c4 + 5, :].rearrange("p a b -> p (a b)")
                        sd = mlp.tile([128, CH], f32, tag="sd")
                        nc.vector.tensor_sub(sd, zpv, mean[:, hf, :])
                        sm = mlp.tile([128, CH], f32, tag="sm")
                        nc.vector.tensor_mul(sm, sd, elv[:, hf, :])
                        nc.vector.tensor_mul(sm, sm, sm)
                        nc.vector.tensor_reduce(out=rtmp, in_=sm, axis=AX.X, op=ALU.add)
                        nc.vector.tensor_add(acc[:, 2:3], acc[:, 2:3], rtmp)

        nc.sync.dma_start(out=d["accs"][:, :], in_=acc)
    nc.finalize()
    return nc


def _prep_inmaps(inputs):
    g = {k: np.asarray(v, dtype=np.float32) for k, v in inputs.items()}
    nf = g["nf_input"].reshape(B, T, 2 * Z)

    # flow weight blob
    wf = np.zeros((128, 10 * WAB_COLS), np.float32)
    bf = np.zeros((128, NL * 20), np.float32)
    for j, li in enumerate([8, 7, 6, 5, 4, 3, 2, 1, 0]):
        Wcat = np.concatenate([g["sW1"][li], g["tW1"][li]], 0)  # [1024, 512]
        if li == 0:
            kts = [Wcat[:, 0:256:2], Wcat[:, 1:256:2], Wcat[:, 256::2], Wcat[:, 257::2]]
            w1off, w2soff, w2toff = 0, 4096, 6144
        else:
            kept, _ = _layer_masks(li)
            kts = [Wcat[:, kept], Wcat[:, 256 + EV], Wcat[:, 256 + OD]]
            w1off, w2soff, w2toff = 0, 3072, 5120
        base = j * WAB_COLS
        for k, kk in enumerate(kts):
            wf[:, base + w1off + k * 1024: base + w1off + (k + 1) * 1024] = kk.T
        for k in range(4):
            wf[:, base + w2soff + k * 512: base + w2soff + (k + 1) * 512] = \
                g["sW2"][li].T[k * 128:(k + 1) * 128, :]
            wf[:, base + w2toff + k * 512: base + w2toff + (k + 1) * 512] = \
                g["tW2"][li].T[k * 128:(k + 1) * 128, :]
        if li == 0:
            W3s = g["sW3"][0][PERM_EO, :]   # [256, 512]
            W3t = g["tW3"][0][PERM_EO, :]
            b3s = g["sb3"][0][PERM_EO]
            b3t = g["tb3"][0][PERM_EO]
            base2 = 9 * WAB_COLS
            for k in range(4):
                wf[:, base2 + k * 256: base2 + (k + 1) * 256] = W3s.T[k * 128:(k + 1) * 128, :]
                wf[:, base2 + 1024 + k * 256: base2 + 1024 + (k + 1) * 256] = \
                    W3t.T[k * 128:(k + 1) * 128, :]
            bf[:, j * 20 + 16] = b3s[0:128]
            bf[:, j * 20 + 17] = b3s[128:256]
            bf[:, j * 20 + 18] = b3t[0:128]
            bf[:, j * 20 + 19] = b3t[128:256]
        else:
            _, upd = _layer_masks(li)
            W3s = g["sW3"][li][upd, :]      # [128, 512]
            W3t = g["tW3"][li][upd, :]
            for k in range(4):
                wf[:, base + 7168 + k * 128: base + 7168 + (k + 1) * 128] = \
                    W3s.T[k * 128:(k + 1) * 128, :]
                wf[:, base + 7680 + k * 128: base + 7680 + (k + 1) * 128] = \
                    W3t.T[k * 128:(k + 1) * 128, :]
            bf[:, j * 20 + 16] = g["sb3"][li][upd]
            bf[:, j * 20 + 18] = g["tb3"][li][upd]
        b1cat = np.concatenate([g["sb1"][li], g["tb1"][li]])
        bf[:, j * 20:j * 20 + 8] = b1cat.reshape(8, 128).T
        bf[:, j * 20 + 8:j * 20 + 12] = g["sb2"][li].reshape(4, 128).T
        bf[:, j * 20 + 12:j * 20 + 16] = g["tb2"][li].reshape(4, 128).T

    # LSTM blob
    wlb = np.zeros((128, WL_COLS), np.float32)

    def put_kt(off, mat_t, width):
        # mat_t: [K, width] -> kt-major tiles [128, width]
        nkt = mat_t.shape[0] // 128
        for k in range(nkt):
            wlb[:, off + k * width: off + (k + 1) * width] = mat_t[k * 128:(k + 1) * 128, :]

    mtp = np.concatenate([np.arange(b * 128, (b + 1) * 128)
                          for b in (0, 1, 4, 5, 2, 3, 6, 7, 8, 9, 10, 11, 12, 13, 14, 15)])
    put_kt(OW_IH1, g["l1Wih"][mtp, :].T[PERM_EO, :], 2048)
    put_kt(OW_HH1, g["l1Whh"][mtp, :].T, 2048)
    put_kt(OW_IH2, g["l2Wih"][mtp, :].T, 2048)
    put_kt(OW_HH2, g["l2Whh"][mtp, :].T, 2048)
    Wml = np.concatenate([g["mW"][PERM_EO, :], g["lvW"][PERM_EO, :]], 0)  # [512, 512]
    put_kt(OW_ML, Wml.T, 512)
    wlb[:, OB1:OB1 + 2048] = np.broadcast_to(
        (g["l1bih"] + g["l1bhh"])[mtp][None, :] / 128.0, (128, 2048))
    wlb[:, OB2:OB2 + 2048] = np.broadcast_to(
        (g["l2bih"] + g["l2bhh"])[mtp][None, :] / 128.0, (128, 2048))
    bml = np.concatenate([g["mb"][PERM_EO], g["lvb"][PERM_EO]])
    wlb[:, OBML:OBML + 512] = np.broadcast_to(bml[None, :] / 128.0, (128, 512))

    import ml_dtypes
    wlb = wlb.astype(ml_dtypes.bfloat16)

    in_maps = []
    for cidx in range(NCORES):
        nfc = nf[cidx * BL:(cidx + 1) * BL]              # [BL, T, 512]
        x = nfc[:, :, Z:]
        p = nfc[:, :, :Z]
        # feature-major [128, C]: partition = channel pair index, col = s*16+t
        m = {
            "xe": np.ascontiguousarray(x[:, :, 0::2].transpose(2, 0, 1).reshape(128, C)),
            "xo": np.ascontiguousarray(x[:, :, 1::2].transpose(2, 0, 1).reshape(128, C)),
            "pe": np.ascontiguousarray(p[:, :, 0::2].transpose(2, 0, 1).reshape(128, C)),
            "po": np.ascontiguousarray(p[:, :, 1::2].transpose(2, 0, 1).reshape(128, C)),
            "wf": wf, "bf": bf, "wl": wlb,
        }
        in_maps.append(m)
    return in_maps


def kernel(**inputs):
    if "nc" not in _CACHE:
        _CACHE["nc"] = _build_nc()
    nc = _CACHE["nc"]
    in_maps = _prep_inmaps(inputs)
    res_obj = run_bass_kernel_spmd(nc, in_maps, list(range(NCORES)),
                                   trace=bool(os.environ.get("KERNEL_TRACE")))
    _CACHE["last"] = res_obj
    res = res_obj.results

    z3 = np.empty((B, T, Z), np.float32)
    s_s = s_z2 = s_sp2 = s_lv = 0.0
    for cidx in range(NCORES):
        r = res[cidx]
        ze = r["z3e"].reshape(128, BL, T)
        zo = r["z3o"].reshape(128, BL, T)
        z3[cidx * BL:(cidx + 1) * BL, :, 0::2] = ze.transpose(1, 2, 0)
        z3[cidx * BL:(cidx + 1) * BL, :, 1::2] = zo.transpose(1, 2, 0)
        a = r["accs"].astype(np.float64)
        s_s += a[:, 0].sum()
        s_z2 += a[:, 1].sum()
        s_sp2 += a[:, 2].sum()
        s_lv += a[:, 3].sum()

    n_el = float(B) * T * Z
    sum_ldj = -s_s
    sum_logN_z3 = -0.5 * s_z2 - n_el * LOG_SQRT_2PI
    sum_logN_samp = -0.5 * s_sp2 - n_el * LOG_SQRT_2PI
    sum_logp2 = -s_lv
    loglik = (sum_logN_samp + sum_logp2) * 0.0025 + (sum_logN_z3 + sum_ldj)
    loss_q = np.float32(-loglik / B)
    return loss_q, z3
